# revision 3
# baseline (speedup 1.0000x reference)
"""Trainium2 Bass kernel for nn_EvoBinarizedLayer.

Math: out[p,b,o] = sum_i x[p,b,i]*w[0,p,i,o] + (1-x[p,b,i])*w[1,p,i,o]
                 = (x @ D)[p,b,o] + colsum(W1)[p,o],   D = W0 - W1

All inputs are {0,1}-valued f32, so D is {-1,0,1} and both x and D are
exactly representable in bf16; PSUM accumulates in fp32, so the bf16
matmul path is bit-exact. colsum(W1) is folded in as:
  - per-i-tile partial sums s[i_local,o] (values <= 8, bf16-exact) on DVE
  - bias[b,o] = ones[128,128].T @ s  (one matmul), added during PSUM evac.

Sharding: population dim P=32 split across 8 cores (4 each), no
cross-core communication.
"""

import numpy as np

P, B, I, O = 32, 512, 1024, 1024
NCORES = 8
PPC = P // NCORES  # populations per core
NIT = I // 128     # i-tiles (contraction)
NBT = B // 128     # b-tiles
NOH = O // 512     # o-halves (PSUM bank width)

_cache = {}


def _patch_tile_drain():
    """This container's walrus caps sem-waits per TPB_CTRL instruction below
    what Tile's final drain needs; spread the waits across nop instructions."""
    import concourse.tile as tile
    import bass_rust
    from concourse.vector_clock import ScopedClock

    if getattr(tile.TileContext, "_drain_patched", False):
        return

    def _drain_and_barrier(self, tick_clock, wait_clock):
        nc = self.nc
        drain_inst = nc.sync.drain()
        wait_clock.add_sem_waits(
            drain_inst.ins, ScopedClock({None: tick_clock.global_clock})
        )
        si = drain_inst.ins.sync_info
        waits = list(si.on_wait or [])
        if len(waits) > 1:
            si.on_wait = waits[:1]
            drain_inst.ins.sync_info = si
            for i in range(1, len(waits)):
                nop = nc.sync.nop()
                nop.ins.sync_info = bass_rust.SyncInfo(
                    on_wait=[waits[i]], on_update=[]
                )
        nc.all_engine_barrier()
        assert self.sems is not None
        popped = nc._tile_sem_poison_stack.pop()
        assert popped is self._sem_poison
        nc.clear_and_free_semaphores(list(self.sems.allocated().values()))
        nc.all_engine_barrier()

    tile.TileContext._drain_and_barrier = _drain_and_barrier
    tile.TileContext._drain_patched = True


MAX_WAITS_PER_INST = 1


def _split_excess_waits(nc):
    """This container's walrus rejects instructions carrying more than a
    couple of sem-waits; hoist excess waits onto same-engine nops placed
    just before the instruction."""
    import concourse.mybir as mybir
    import bass_rust

    n_split = 0
    for fn in nc.m.functions:
        for bb in fn.blocks:
            new_insts = []
            for inst in bb.instructions:
                si = inst.sync_info
                waits = list(si.on_wait) if si and si.on_wait else []
                if len(waits) > MAX_WAITS_PER_INST:
                    n_split += 1
                    extra = waits[: -MAX_WAITS_PER_INST]
                    keep = waits[-MAX_WAITS_PER_INST:]
                    for j in range(0, len(extra), MAX_WAITS_PER_INST):
                        nop = mybir.InstNoOp(
                            name=nc.get_next_instruction_name(), ins=[], outs=[]
                        )
                        nop.engine = inst.engine
                        nop.sync_info = bass_rust.SyncInfo(
                            on_wait=extra[j : j + MAX_WAITS_PER_INST], on_update=[]
                        )
                        nc.register_instruction(nop, overwrite=True)
                        new_insts.append(nop)
                    si.on_wait = keep
                    inst.sync_info = si
                new_insts.append(inst)
            bb.instructions = new_insts
    return n_split


def _build_nc():
    from contextlib import ExitStack

    import concourse.bass as bass
    import concourse.mybir as mybir
    import concourse.tile as tile
    from concourse.masks import make_identity

    _patch_tile_drain()

    f32 = mybir.dt.float32
    bf16 = mybir.dt.bfloat16

    nc = bass.Bass()
    x_in = nc.declare_dram_parameter("x", [PPC, B, I], f32, isOutput=False)
    w_in = nc.declare_dram_parameter("w", [2, PPC, I, O], f32, isOutput=False)
    out_ext = nc.declare_dram_parameter("out", [PPC, B, O], f32, isOutput=True)

    with ExitStack() as ctx:
        tc = ctx.enter_context(tile.TileContext(nc))
        const_pool = ctx.enter_context(tc.tile_pool(name="const", bufs=1))
        w_pool = ctx.enter_context(tc.tile_pool(name="w", bufs=4))
        d_pool = ctx.enter_context(tc.tile_pool(name="d", bufs=2))
        s_pool = ctx.enter_context(tc.tile_pool(name="s", bufs=2))
        x_pool = ctx.enter_context(tc.tile_pool(name="xp", bufs=4))
        xt_pool = ctx.enter_context(tc.tile_pool(name="xt", bufs=2))
        bias_pool = ctx.enter_context(tc.tile_pool(name="bias", bufs=2))
        out_pool = ctx.enter_context(tc.tile_pool(name="op", bufs=4))
        psum_mm = ctx.enter_context(tc.tile_pool(name="pmm", bufs=5, space="PSUM"))
        psum_tr = ctx.enter_context(tc.tile_pool(name="ptr", bufs=3, space="PSUM"))

        ident = const_pool.tile([128, 128], f32)
        make_identity(nc, ident[:])
        ones_bf = const_pool.tile([128, 128], bf16)
        nc.gpsimd.memset(ones_bf[:], 1.0)

        for p in range(PPC):
            # ---- weights: D = w0 - w1 (bf16), s_bf = per-partition partial colsum(w1)
            d_p = d_pool.tile([128, NIT * O], bf16, name=f"d_{p}", tag="d")
            s_acc = s_pool.tile([128, O], f32, name=f"sacc_{p}", tag="sacc")
            s_bf = s_pool.tile([128, O], bf16, name=f"sbf_{p}", tag="sbf")
            for it in range(NIT):
                w0_t = w_pool.tile([128, O], f32, name=f"w0_{p}_{it}", tag="w0")
                w1_t = w_pool.tile([128, O], f32, name=f"w1_{p}_{it}", tag="w1")
                nc.sync.dma_start(w0_t[:], w_in[0, p, it * 128 : (it + 1) * 128, :])
                nc.sync.dma_start(w1_t[:], w_in[1, p, it * 128 : (it + 1) * 128, :])
                nc.vector.tensor_sub(
                    d_p[:, it * O : (it + 1) * O], w0_t[:], w1_t[:]
                )
                if it == 0:
                    nc.vector.tensor_copy(s_acc[:], w1_t[:])
                elif it < NIT - 1:
                    nc.vector.tensor_add(s_acc[:], s_acc[:], w1_t[:])
                else:
                    nc.vector.tensor_add(s_bf[:], s_acc[:], w1_t[:])

            # ---- bias[b, o] = sum_k s_bf[k, o] (same for every b row)
            bias_sb = bias_pool.tile([128, O], f32, name=f"bias_{p}", tag="bias")
            for oh in range(NOH):
                pb = psum_mm.tile([128, 512], f32, name=f"pb_{p}_{oh}", tag="g")
                nc.tensor.matmul(
                    pb[:],
                    ones_bf[:],
                    s_bf[:, oh * 512 : (oh + 1) * 512],
                    start=True,
                    stop=True,
                )
                nc.scalar.copy(bias_sb[:, oh * 512 : (oh + 1) * 512], pb[:])

            # ---- x: load, PE-transpose into bf16 xT tiles
            xT_p = xt_pool.tile([128, NBT * I], bf16, name=f"xT_{p}", tag="xT")
            for bt in range(NBT):
                x_t = x_pool.tile([128, I], f32, name=f"x_{p}_{bt}", tag="x")
                nc.sync.dma_start(x_t[:], x_in[p, bt * 128 : (bt + 1) * 128, :])
                for g in range(2):
                    ptr = psum_tr.tile(
                        [128, 512], f32, name=f"ptr_{p}_{bt}_{g}", tag="tr"
                    )
                    for c in range(4):
                        it = g * 4 + c
                        nc.tensor.transpose(
                            ptr[:, c * 128 : (c + 1) * 128],
                            x_t[:, it * 128 : (it + 1) * 128],
                            ident[:],
                        )
                    nc.scalar.copy(
                        xT_p[:, bt * I + g * 512 : bt * I + (g + 1) * 512], ptr[:]
                    )

            # ---- main matmuls + bias add on evac
            for bt in range(NBT):
                out_sb = out_pool.tile([128, O], f32, name=f"out_{p}_{bt}", tag="out")
                for oh in range(NOH):
                    pmm = psum_mm.tile(
                        [128, 512], f32, name=f"pmm_{p}_{bt}_{oh}", tag="g"
                    )
                    for it in range(NIT):
                        nc.tensor.matmul(
                            pmm[:],
                            xT_p[:, bt * I + it * 128 : bt * I + (it + 1) * 128],
                            d_p[:, it * O + oh * 512 : it * O + oh * 512 + 512],
                            start=(it == 0),
                            stop=(it == NIT - 1),
                        )
                    nc.vector.tensor_add(
                        out_sb[:, oh * 512 : (oh + 1) * 512],
                        pmm[:],
                        bias_sb[:, oh * 512 : (oh + 1) * 512],
                    )
                nc.sync.dma_start(
                    out_ext[p, bt * 128 : (bt + 1) * 128, :], out_sb[:]
                )

    _split_excess_waits(nc)
    return nc


def get_nc():
    if "nc" not in _cache:
        _cache["nc"] = _build_nc()
    return _cache["nc"]


def run(x, w, trace=False, **kwargs):
    from concourse.bass_utils import run_bass_kernel_spmd

    x = np.ascontiguousarray(np.asarray(x, dtype=np.float32))
    w = np.ascontiguousarray(np.asarray(w, dtype=np.float32))
    assert x.shape == (P, B, I) and w.shape == (2, P, I, O)

    nc = get_nc()
    in_maps = [
        {
            "x": np.ascontiguousarray(x[c * PPC : (c + 1) * PPC]),
            "w": np.ascontiguousarray(w[:, c * PPC : (c + 1) * PPC]),
        }
        for c in range(NCORES)
    ]
    res = run_bass_kernel_spmd(nc, in_maps, list(range(NCORES)), trace=trace, **kwargs)
    out = np.concatenate([res.results[c]["out"] for c in range(NCORES)], axis=0)
    return out.astype(np.float32, copy=False), res


def kernel(x, w):
    out, _ = run(x, w, trace=False)
    return out
